# revision 1
# baseline (speedup 1.0000x reference)
"""Trainium2 8-core GATv2 message-passing kernel (nn_AtomGraphEncoder).

Design:
- Nodes block-sharded 8x12500; edges assigned to dst's core.
- Per layer: proj fs/fd (PE, bf16), AllGather fs table, dma_gather rows
  (4 src-classes of 32768 for int16 idx), ACT Prelu + sign-split reduces
  for GATv2 logits (|a| folded into W columns on host), exp, msg = ex*fs,
  dma_scatter_add of [msg|ex|0] (elem 384 bf16) into per-core accumulator
  with round-split calls (unique dst per call; dummies -> trash row),
  readback + normalize + PE-transpose into hT for the next layer.
- Host: weight folding/permutation (undone on output), uniform-across-cores
  slot structure (SPMD single graph).
"""
import sys

import numpy as np
import ml_dtypes

sys.path.insert(0, '/opt/trn_rl_repo')

N, E = 100000, 400000
ATOM_DIM, HID, LAYERS, HEADS = 74, 256, 3, 4
OUT = HID // HEADS
NCORES = 8
NPC = N // NCORES           # 12500
CLS, CLSR = 4, 32768
TRASH = NPC                 # scatter trash row
NACC = 12544                # 98*128 accumulator rows
PAY = 384                   # scatter elem (bf16): [msg 256 | ex 4 | zeros]
CHUNK = 8                   # tile-columns per pipeline chunk (1024 slots)
BF = ml_dtypes.bfloat16
EPS = 1e-20


def _bf(x):
    return np.asarray(x).astype(BF)


def _fold_weights(W_in, b_in, W_src, b_src, W_dst, b_dst, attn, bias):
    Ts, Tinvs = [], []
    pos_cnt = np.zeros((LAYERS, HEADS), np.int64)
    zero_cnt = np.zeros((LAYERS, HEADS), np.int64)
    for l in range(LAYERS):
        Tl = np.zeros((HID, HID), np.float64)
        Tinv = np.zeros((HID, HID), np.float64)
        for h in range(HEADS):
            a = np.asarray(attn)[l, h].astype(np.float64)
            order = np.concatenate([
                np.where(a > 0)[0], np.where(a == 0)[0], np.where(a < 0)[0]])
            pos_cnt[l, h] = (a > 0).sum()
            zero_cnt[l, h] = (a == 0).sum()
            for j, p in enumerate(order):
                s = abs(a[p]) if a[p] != 0 else 1.0
                Tl[h * OUT + p, h * OUT + j] = s
                Tinv[h * OUT + j, h * OUT + p] = 1.0 / s
        Ts.append(Tl)
        Tinvs.append(Tinv)
    Ws_eff, Wd_eff, bs_eff, bd_eff = [], [], [], []
    for l in range(LAYERS):
        Tp = np.eye(HID) if l == 0 else Tinvs[l - 1]
        Ws = np.asarray(W_src)[l].astype(np.float64)
        Wd = np.asarray(W_dst)[l].astype(np.float64)
        bprev = np.zeros(HID) if l == 0 else np.asarray(bias)[l - 1].astype(np.float64)
        Ws_eff.append((Tp @ Ws @ Ts[l]).astype(np.float32))
        Wd_eff.append((Tp @ Wd @ Ts[l]).astype(np.float32))
        bs_eff.append(((np.asarray(b_src)[l] + bprev @ Ws) @ Ts[l]).astype(np.float32))
        bd_eff.append(((np.asarray(b_dst)[l] + bprev @ Wd) @ Ts[l]).astype(np.float32))
    return Ws_eff, Wd_eff, bs_eff, bd_eff, pos_cnt, zero_cnt, Tinvs[-1]


def _prep_slots(src, dst):
    """Uniform slot structure. Returns per-core idx arrays + call metadata."""
    src = np.asarray(src).astype(np.int64)
    dst = np.asarray(dst).astype(np.int64)
    percore = []
    for c in range(NCORES):
        m = (dst >= c * NPC) & (dst < (c + 1) * NPC)
        es, ed = src[m], dst[m] - c * NPC
        order = np.lexsort((es, ed))
        es, ed = es[order], ed[order]
        cnt = np.bincount(ed, minlength=NPC)
        starts = np.concatenate([[0], np.cumsum(cnt)])
        rank = np.arange(len(ed)) - starts[ed]
        percore.append((es, ed, rank, es // CLSR))
    R = max(int(pc[2].max()) + 1 for pc in percore) if E else 1

    # uniform segment sizes per (round, class)
    seg = np.zeros((R, CLS), np.int64)
    for c in range(NCORES):
        es, ed, rank, cls = percore[c]
        for r in range(R):
            for k in range(CLS):
                seg[r, k] = max(seg[r, k], int(((rank == r) & (cls == k)).sum()))
    seg = ((seg + 127) // 128) * 128
    S = int(seg.sum())
    T_all = S // 128

    # segment boundary tables (slot offsets), identical across cores
    fs_segs = []   # (slot_off, n, class)
    sc_segs = []   # (slot_off, n) per round
    off = 0
    for r in range(R):
        r_off = off
        for k in range(CLS):
            n = int(seg[r, k])
            if n:
                fs_segs.append((off, n, k))
            off += n
        if off > r_off:
            sc_segs.append((r_off, off - r_off))

    cores = []
    for c in range(NCORES):
        es, ed, rank, cls = percore[c]
        fs_idx = np.zeros(S, np.int64)
        fd_idx = np.zeros(S, np.int64)
        sc_idx = np.full(S, TRASH, np.int64)
        off = 0
        for r in range(R):
            for k in range(CLS):
                n = int(seg[r, k])
                if n == 0:
                    continue
                m = (rank == r) & (cls == k)
                cntm = int(m.sum())
                fs_idx[off:off + cntm] = es[m] - k * CLSR
                fd_idx[off:off + cntm] = ed[m]
                sc_idx[off:off + cntm] = ed[m]
                off += n
        cores.append((fs_idx, fd_idx, sc_idx))
    return cores, fs_segs, sc_segs, S, T_all


def _wrap16(idx):
    w = np.ascontiguousarray(np.asarray(idx).reshape(-1, 16).T).astype(np.int16)
    return np.tile(w, (8, 1))


def _pieces(segs, t0, t1):
    """Intersect segment list [(off, n, *rest)] with slot range [t0*128, t1*128)."""
    a, b = t0 * 128, t1 * 128
    out = []
    for s in segs:
        off, n = s[0], s[1]
        lo, hi = max(off, a), min(off + n, b)
        if lo < hi:
            out.append((lo, hi - lo) + tuple(s[2:]))
    return out


def _build(prep, pos_cnt, zero_cnt):
    import concourse.bass as bass
    import concourse.tile as tile
    from concourse import bacc, mybir, library_config

    cores, fs_segs, sc_segs, S, T_all = prep
    ALPHA = 0.2

    nc = bacc.Bacc("TRN2", target_bir_lowering=False, debug=False,
                   num_devices=NCORES)
    dt = mybir.dt
    atomT_d = nc.dram_tensor("atomT", [ATOM_DIM + 1, NPC], dt.bfloat16,
                             kind="ExternalInput")
    win_d = nc.dram_tensor("win", [ATOM_DIM + 1, HID], dt.bfloat16,
                           kind="ExternalInput")
    wsd_d = nc.dram_tensor("wsd", [128, 2 * LAYERS, 512], dt.bfloat16,
                           kind="ExternalInput")
    fsi_d = nc.dram_tensor("fsi", [128, S // 16], dt.int16, kind="ExternalInput")
    fdi_d = nc.dram_tensor("fdi", [128, S // 16], dt.int16, kind="ExternalInput")
    sci_d = nc.dram_tensor("sci", [128, S // 16], dt.int16, kind="ExternalInput")
    ident_d = nc.dram_tensor("ident", [128, 128], dt.bfloat16, kind="ExternalInput")
    out_d = nc.dram_tensor("out", [NPC, HID], dt.float32, kind="ExternalOutput")

    fs_bounce = nc.dram_tensor("fs_bounce", [NPC, HID], dt.bfloat16)
    fs_full = nc.dram_tensor("fs_full", [N, HID], dt.bfloat16)
    fd_tab = nc.dram_tensor("fd_tab", [NPC, HID], dt.bfloat16)
    rst = nc.dram_tensor("rst", [NACC, PAY], dt.bfloat16)
    rst2 = nc.dram_tensor("rst2", [NACC, PAY], dt.bfloat16)

    sc_par_segs = [(off, n, i % 2) for i, (off, n) in enumerate(sc_segs)]
    NB = 98            # 128-node blocks (last has 84 valid)
    n_chunks = (T_all + CHUNK - 1) // CHUNK

    with tile.TileContext(nc) as tc:
        nc.gpsimd.load_library(library_config.mlp)
        with tc.tile_pool(name="persist", bufs=1) as pp, \
             tc.tile_pool(name="work", bufs=3) as wp, \
             tc.tile_pool(name="payp", bufs=4) as yp, \
             tc.tile_pool(name="mono", bufs=1) as mp, \
             tc.tile_pool(name="stage", bufs=2) as sp, \
             tc.tile_pool(name="psA", bufs=2, space="PSUM") as psA, \
             tc.tile_pool(name="psT", bufs=2, space="PSUM") as psT:

            fsi = pp.tile([128, S // 16], dt.int16, tag="fsi")
            fdi = pp.tile([128, S // 16], dt.int16, tag="fdi")
            sci = pp.tile([128, S // 16], dt.int16, tag="sci")
            wsd = pp.tile([128, 2 * LAYERS, 512], dt.bfloat16, tag="wsd")
            win = pp.tile([ATOM_DIM + 1, HID], dt.bfloat16, tag="win")
            ident = pp.tile([128, 128], dt.bfloat16, tag="ident")
            hT = pp.tile([128, 2, NB * 128], dt.bfloat16, tag="hT")
            nc.sync.dma_start(fsi[:], fsi_d[:])
            nc.sync.dma_start(fdi[:], fdi_d[:])
            nc.sync.dma_start(sci[:], sci_d[:])
            nc.sync.dma_start(wsd[:], wsd_d[:])
            nc.sync.dma_start(win[:], win_d[:])
            nc.sync.dma_start(ident[:], ident_d[:])

            # ---- input projection: hT0 = (atomT^T @ Win)^T directly
            # atomT shares its memory (tag) with zpay: atomT is dead after
            # the input projection, before the first zpay memset runs.
            atomT = mp.tile([ATOM_DIM + 1, NPC], dt.bfloat16, tag="big")
            nc.sync.dma_start(atomT[:], atomT_d[:])
            NS = 25
            W_ = NPC // NS  # 500
            for s in range(NS):
                for cch in range(2):
                    ps = psA.tile([128, 512], dt.float32, tag="projps")
                    nc.tensor.matmul(ps[:, 0:W_],
                                     win[:, cch * 128:(cch + 1) * 128],
                                     atomT[:, s * W_:(s + 1) * W_],
                                     start=True, stop=True)
                    nc.scalar.activation(out=hT[:, cch, s * W_:(s + 1) * W_],
                                         in_=ps[:, 0:W_],
                                         func=mybir.ActivationFunctionType.Copy)

            zpay = mp.tile([128, 14, PAY], dt.bfloat16, tag="big")
            nc.vector.memset(zpay[:], 0.0)

            for l in range(LAYERS):
                last = l == LAYERS - 1
                # ---- projection: fs/fd tables
                for a in range(NB):
                    nt = 128 if a < NB - 1 else NPC - 128 * (NB - 1)
                    j = a % 8
                    if j == 0:
                        fs_sb = sp.tile([128, 8, HID], dt.bfloat16, tag="fs_sb")
                        fd_sb = sp.tile([128, 8, HID], dt.bfloat16, tag="fd_sb")
                    ps = psA.tile([128, 512], dt.float32, tag="projps")
                    for kc in range(2):
                        nc.tensor.matmul(
                            ps[0:nt, :],
                            hT[:, kc, a * 128:a * 128 + nt],
                            wsd[:, l * 2 + kc, :],
                            start=(kc == 0), stop=(kc == 1))
                    nc.scalar.activation(out=fs_sb[0:nt, j, :], in_=ps[0:nt, 0:HID],
                                         func=mybir.ActivationFunctionType.Copy)
                    nc.scalar.activation(out=fd_sb[0:nt, j, :], in_=ps[0:nt, HID:512],
                                         func=mybir.ActivationFunctionType.Copy)
                    if j == 7 or a == NB - 1:
                        a0 = a - j
                        fullc = j + (1 if nt == 128 else 0)
                        if fullc:
                            nc.sync.dma_start(
                                fs_bounce[a0 * 128:(a0 + fullc) * 128, :].rearrange(
                                    "(a p) e -> p a e", p=128),
                                fs_sb[:, 0:fullc, :])
                            nc.sync.dma_start(
                                fd_tab[a0 * 128:(a0 + fullc) * 128, :].rearrange(
                                    "(a p) e -> p a e", p=128),
                                fd_sb[:, 0:fullc, :])
                        if nt < 128:
                            nc.sync.dma_start(
                                fs_bounce[(NB - 1) * 128:NPC, :],
                                fs_sb[0:nt, j, :])
                            nc.sync.dma_start(
                                fd_tab[(NB - 1) * 128:NPC, :],
                                fd_sb[0:nt, j, :])

                # ---- AllGather fs table
                nc.gpsimd.collective_compute(
                    "AllGather", mybir.AluOpType.bypass,
                    replica_groups=[list(range(NCORES))],
                    ins=[fs_bounce[:].opt()], outs=[fs_full[:].opt()])

                # ---- zero-init accumulators (98 = 7 x 14 column groups)
                for acc in (rst, rst2):
                    for zk in range(7):
                        nc.sync.dma_start(
                            acc[zk * 14 * 128:(zk + 1) * 14 * 128, :].rearrange(
                                "(a p) e -> p a e", p=128), zpay[:])

                # ---- per-chunk edge pipeline
                for ch in range(n_chunks):
                    t0 = ch * CHUNK
                    t1 = min(t0 + CHUNK, T_all)
                    tcn = t1 - t0
                    fsg = wp.tile([128, CHUNK, HID], dt.bfloat16, tag="fsg")
                    fdg = wp.tile([128, CHUNK, HID], dt.bfloat16, tag="fdg")
                    pay = yp.tile([128, CHUNK, PAY], dt.bfloat16, tag="pay")
                    pn = wp.tile([128, CHUNK, 8], dt.float32, tag="pn")
                    lg = wp.tile([128, CHUNK, 4], dt.float32, tag="lg")
                    exb = wp.tile([128, CHUNK, 4], dt.bfloat16, tag="exb")

                    for (so, n, k) in _pieces(fs_segs, t0, t1):
                        hi = min((k + 1) * CLSR, N)
                        nc.gpsimd.dma_gather(
                            fsg[:, so // 128 - t0:(so + n) // 128 - t0, :],
                            fs_full[k * CLSR:hi, :],
                            fsi[:, so // 16:(so + n) // 16], n, n, HID)
                    nc.gpsimd.dma_gather(
                        fdg[:, 0:tcn, :], fd_tab[:],
                        fdi[:, t0 * 8:t1 * 8], tcn * 128, tcn * 128, HID)

                    u = fdg  # in-place: u = lrelu(fsg + fdg)
                    nc.vector.tensor_tensor(out=u[:, 0:tcn, :], in0=fsg[:, 0:tcn, :],
                                            in1=fdg[:, 0:tcn, :],
                                            op=mybir.AluOpType.add)
                    nc.scalar.activation(out=u[:, 0:tcn, :], in_=u[:, 0:tcn, :],
                                         func=mybir.ActivationFunctionType.Prelu,
                                         alpha=ALPHA)
                    for h in range(HEADS):
                        kp = int(pos_cnt[l, h])
                        kz = int(zero_cnt[l, h])
                        if kp > 0:
                            nc.vector.tensor_reduce(
                                out=pn[:, 0:tcn, h],
                                in_=u[:, 0:tcn, h * OUT:h * OUT + kp],
                                axis=mybir.AxisListType.X, op=mybir.AluOpType.add)
                        else:
                            nc.vector.memset(pn[:, 0:tcn, h], 0.0)
                        if kp + kz < OUT:
                            nc.vector.tensor_reduce(
                                out=pn[:, 0:tcn, 4 + h],
                                in_=u[:, 0:tcn, h * OUT + kp + kz:(h + 1) * OUT],
                                axis=mybir.AxisListType.X, op=mybir.AluOpType.add)
                        else:
                            nc.vector.memset(pn[:, 0:tcn, 4 + h], 0.0)
                    nc.vector.tensor_tensor(out=lg[:, 0:tcn, :],
                                            in0=pn[:, 0:tcn, 0:4],
                                            in1=pn[:, 0:tcn, 4:8],
                                            op=mybir.AluOpType.subtract)
                    nc.scalar.activation(out=exb[:, 0:tcn, :], in_=lg[:, 0:tcn, :],
                                         func=mybir.ActivationFunctionType.Exp)
                    nc.vector.memset(pay[:, 0:tcn, HID + HEADS:PAY], 0.0)
                    nc.vector.tensor_copy(out=pay[:, 0:tcn, HID:HID + HEADS],
                                          in_=exb[:, 0:tcn, :])
                    nc.vector.tensor_tensor(
                        out=pay[:, 0:tcn, 0:HID].rearrange(
                            "p t (h d) -> p t h d", h=HEADS),
                        in0=fsg[:, 0:tcn, :].rearrange(
                            "p t (h d) -> p t h d", h=HEADS),
                        in1=exb[:, 0:tcn, :].unsqueeze(3).broadcast_to(
                            [128, tcn, HEADS, OUT]),
                        op=mybir.AluOpType.mult)
                    for (so0, n0, rpar) in _pieces(sc_par_segs, t0, t1):
                        acc = rst if rpar == 0 else rst2
                        for so in range(so0, so0 + n0, 640):
                            n = min(640, so0 + n0 - so)
                            nc.gpsimd.dma_scatter_add(
                                acc[:],
                                pay[:, so // 128 - t0:(so + n) // 128 - t0, :],
                                sci[:, so // 16:(so + n) // 16], n, n, PAY)

                # ---- readback + normalize (+ transpose for next layer)
                for g0 in range(0, NB, 8):
                    gn = min(8, NB - g0)
                    rb = sp.tile([128, 8, PAY], dt.bfloat16, tag="fs_sb")
                    rb2 = sp.tile([128, 8, PAY], dt.bfloat16, tag="rb2")
                    nc.sync.dma_start(
                        rb[:, 0:gn, :],
                        rst[g0 * 128:(g0 + gn) * 128, :].rearrange(
                            "(a p) e -> p a e", p=128))
                    nc.sync.dma_start(
                        rb2[:, 0:gn, :],
                        rst2[g0 * 128:(g0 + gn) * 128, :].rearrange(
                            "(a p) e -> p a e", p=128))
                    nc.vector.tensor_tensor(out=rb[:, 0:gn, :], in0=rb[:, 0:gn, :],
                                            in1=rb2[:, 0:gn, :],
                                            op=mybir.AluOpType.add)
                    hn = sp.tile([128, 8, HID],
                                 dt.float32 if last else dt.bfloat16, tag="fd_sb")
                    denf = wp.tile([128, 32], dt.float32, tag="denf")
                    rec = wp.tile([128, 32], dt.float32, tag="rec")
                    nc.vector.tensor_scalar(
                        out=denf[:, 0:gn * 4].rearrange("p (a b) -> p a b", b=4),
                        in0=rb[:, 0:gn, HID:HID + HEADS],
                        scalar1=EPS, scalar2=None, op0=mybir.AluOpType.add)
                    nc.vector.reciprocal(out=rec[:, 0:gn * 4], in_=denf[:, 0:gn * 4])
                    for j in range(gn):
                        for h in range(HEADS):
                            nc.vector.tensor_scalar(
                                out=hn[:, j, h * OUT:(h + 1) * OUT],
                                in0=rb[:, j, h * OUT:(h + 1) * OUT],
                                scalar1=rec[:, j * 4 + h:j * 4 + h + 1], scalar2=None,
                                op0=mybir.AluOpType.mult)
                    if last:
                        nn = min(NPC - g0 * 128, gn * 128)
                        if nn == gn * 128:
                            nc.sync.dma_start(
                                out_d[g0 * 128:g0 * 128 + nn, :].rearrange(
                                    "(a p) e -> p a e", p=128),
                                hn[:, 0:gn, :])
                        else:
                            fullc = nn // 128
                            if fullc:
                                nc.sync.dma_start(
                                    out_d[g0 * 128:(g0 + fullc) * 128, :].rearrange(
                                        "(a p) e -> p a e", p=128),
                                    hn[:, 0:fullc, :])
                            rem = nn - fullc * 128
                            nc.sync.dma_start(
                                out_d[(g0 + fullc) * 128:(g0 + fullc) * 128 + rem, :],
                                hn[0:rem, fullc, :])
                    else:
                        for j in range(gn):
                            a = g0 + j
                            for cch in range(2):
                                pt = psT.tile([128, 128], dt.bfloat16, tag="tp")
                                nc.tensor.transpose(
                                    pt[:], hn[:, j, cch * 128:(cch + 1) * 128],
                                    ident[:])
                                nc.vector.tensor_copy(
                                    out=hT[:, cch, a * 128:(a + 1) * 128],
                                    in_=pt[:])
    nc.compile()
    return nc


def kernel(**inputs):
    from concourse.bass_utils import run_bass_kernel_spmd

    src = np.asarray(inputs['src'])
    dst = np.asarray(inputs['dst'])
    atom = np.asarray(inputs['atom_feat']).astype(np.float32)
    Ws_eff, Wd_eff, bs_eff, bd_eff, pos_cnt, zero_cnt, T2inv = _fold_weights(
        inputs['W_in'], inputs['b_in'], inputs['W_src'], inputs['b_src'],
        inputs['W_dst'], inputs['b_dst'], inputs['attn'], inputs['bias'])
    prep = _prep_slots(src, dst)
    cores, fs_segs, sc_segs, S, T_all = prep

    # weight tensors
    win_np = np.zeros((ATOM_DIM + 1, HID), np.float32)
    win_np[:ATOM_DIM] = np.asarray(inputs['W_in'])
    win_np[ATOM_DIM] = np.asarray(inputs['b_in'])
    wsd_np = np.zeros((128, 2 * LAYERS, 512), np.float32)
    for l in range(LAYERS):
        for kc in range(2):
            wsd_np[:, l * 2 + kc, 0:HID] = Ws_eff[l][kc * 128:(kc + 1) * 128]
            wsd_np[:, l * 2 + kc, HID:512] = Wd_eff[l][kc * 128:(kc + 1) * 128]
    # biases are folded via ones-row only for input proj; bs_eff/bd_eff assumed 0
    # (setup_inputs uses zero biases). Guard:
    for l in range(LAYERS):
        assert np.abs(bs_eff[l]).max() < 1e-12 and np.abs(bd_eff[l]).max() < 1e-12, \
            "nonzero GAT biases not supported by this kernel build"

    nc = _build(prep, pos_cnt, zero_cnt)

    ident = np.eye(128, dtype=np.float32)
    in_maps = []
    for c in range(NCORES):
        fs_idx, fd_idx, sc_idx = cores[c]
        at = np.zeros((ATOM_DIM + 1, NPC), np.float32)
        at[:ATOM_DIM] = atom[c * NPC:(c + 1) * NPC].T
        at[ATOM_DIM] = 1.0
        in_maps.append({
            'atomT': _bf(at), 'win': _bf(win_np), 'wsd': _bf(wsd_np),
            'fsi': _wrap16(fs_idx), 'fdi': _wrap16(fd_idx),
            'sci': _wrap16(sc_idx), 'ident': _bf(ident),
        })
    import os
    res = run_bass_kernel_spmd(nc, in_maps, core_ids=list(range(NCORES)),
                               trace=bool(os.environ.get('KBT_TRACE')))
    kernel._last = res
    out = np.concatenate([res.results[c]['out'] for c in range(NCORES)], 0)
    out = out.astype(np.float64) @ T2inv + np.asarray(inputs['bias'])[LAYERS - 1][None]
    return out.astype(np.float32)


if __name__ == '__main__':
    import jax
    with jax.default_device(jax.devices('cpu')[0]):
        import reference
        inputs = {k: np.asarray(v) for k, v in reference.setup_inputs().items()}
    got = kernel(**inputs)
    print("kernel out:", got.shape, got.dtype, np.abs(got).mean())



# revision 2
# speedup vs baseline: 1.2809x; 1.2809x over previous
"""Trainium2 8-core GATv2 message-passing kernel (nn_AtomGraphEncoder).

Design:
- Nodes block-sharded 8x12500; edges assigned to dst's core.
- Per layer: proj fs/fd (PE, bf16), AllGather fs table, dma_gather rows
  (4 src-classes of 32768 for int16 idx), ACT Prelu + sign-split reduces
  for GATv2 logits (|a| folded into W columns on host), exp, msg = ex*fs,
  dma_scatter_add of [msg|ex|0] (elem 384 bf16) into per-core accumulator
  with round-split calls (unique dst per call; dummies -> trash row),
  readback + normalize + PE-transpose into hT for the next layer.
- Host: weight folding/permutation (undone on output), uniform-across-cores
  slot structure (SPMD single graph).
"""
import sys

import numpy as np
import ml_dtypes

sys.path.insert(0, '/opt/trn_rl_repo')

N, E = 100000, 400000
ATOM_DIM, HID, LAYERS, HEADS = 74, 256, 3, 4
OUT = HID // HEADS
NCORES = 8
NPC = N // NCORES           # 12500
CLS, CLSR = 4, 32768
TRASH = NPC                 # scatter trash row
NACC = 12544                # 98*128 accumulator rows
PAY = 384                   # scatter elem (bf16): [msg 256 | ex 4 | zeros]
CHUNK = 8                   # tile-columns per pipeline chunk (1024 slots)
BF = ml_dtypes.bfloat16
EPS = 1e-20


def _bf(x):
    return np.asarray(x).astype(BF)


def _fold_weights(W_in, b_in, W_src, b_src, W_dst, b_dst, attn, bias):
    Ts, Tinvs = [], []
    pos_cnt = np.zeros((LAYERS, HEADS), np.int64)
    zero_cnt = np.zeros((LAYERS, HEADS), np.int64)
    for l in range(LAYERS):
        Tl = np.zeros((HID, HID), np.float64)
        Tinv = np.zeros((HID, HID), np.float64)
        for h in range(HEADS):
            a = np.asarray(attn)[l, h].astype(np.float64)
            order = np.concatenate([
                np.where(a > 0)[0], np.where(a == 0)[0], np.where(a < 0)[0]])
            pos_cnt[l, h] = (a > 0).sum()
            zero_cnt[l, h] = (a == 0).sum()
            for j, p in enumerate(order):
                s = abs(a[p]) if a[p] != 0 else 1.0
                Tl[h * OUT + p, h * OUT + j] = s
                Tinv[h * OUT + j, h * OUT + p] = 1.0 / s
        Ts.append(Tl)
        Tinvs.append(Tinv)
    Ws_eff, Wd_eff, bs_eff, bd_eff = [], [], [], []
    for l in range(LAYERS):
        Tp = np.eye(HID) if l == 0 else Tinvs[l - 1]
        Ws = np.asarray(W_src)[l].astype(np.float64)
        Wd = np.asarray(W_dst)[l].astype(np.float64)
        bprev = np.zeros(HID) if l == 0 else np.asarray(bias)[l - 1].astype(np.float64)
        Ws_eff.append((Tp @ Ws @ Ts[l]).astype(np.float32))
        Wd_eff.append((Tp @ Wd @ Ts[l]).astype(np.float32))
        bs_eff.append(((np.asarray(b_src)[l] + bprev @ Ws) @ Ts[l]).astype(np.float32))
        bd_eff.append(((np.asarray(b_dst)[l] + bprev @ Wd) @ Ts[l]).astype(np.float32))
    return Ws_eff, Wd_eff, bs_eff, bd_eff, pos_cnt, zero_cnt, Tinvs[-1]


def _prep_slots(src, dst):
    """Uniform slot structure. Returns per-core idx arrays + call metadata."""
    src = np.asarray(src).astype(np.int64)
    dst = np.asarray(dst).astype(np.int64)
    percore = []
    for c in range(NCORES):
        m = (dst >= c * NPC) & (dst < (c + 1) * NPC)
        es, ed = src[m], dst[m] - c * NPC
        order = np.lexsort((es, ed))
        es, ed = es[order], ed[order]
        cnt = np.bincount(ed, minlength=NPC)
        starts = np.concatenate([[0], np.cumsum(cnt)])
        rank = np.arange(len(ed)) - starts[ed]
        percore.append((es, ed, rank, es // CLSR))
    R = max(int(pc[2].max()) + 1 for pc in percore) if E else 1

    # uniform segment sizes per (round, class)
    seg = np.zeros((R, CLS), np.int64)
    for c in range(NCORES):
        es, ed, rank, cls = percore[c]
        for r in range(R):
            for k in range(CLS):
                seg[r, k] = max(seg[r, k], int(((rank == r) & (cls == k)).sum()))
    seg = ((seg + 127) // 128) * 128
    S = int(seg.sum())
    T_all = S // 128

    # segment boundary tables (slot offsets), identical across cores
    fs_segs = []   # (slot_off, n, class)
    sc_segs = []   # (slot_off, n) per round
    off = 0
    for r in range(R):
        r_off = off
        for k in range(CLS):
            n = int(seg[r, k])
            if n:
                fs_segs.append((off, n, k))
            off += n
        if off > r_off:
            sc_segs.append((r_off, off - r_off))

    cores = []
    for c in range(NCORES):
        es, ed, rank, cls = percore[c]
        fs_idx = np.zeros(S, np.int64)
        fd_idx = np.zeros(S, np.int64)
        sc_idx = np.full(S, TRASH, np.int64)
        off = 0
        for r in range(R):
            for k in range(CLS):
                n = int(seg[r, k])
                if n == 0:
                    continue
                m = (rank == r) & (cls == k)
                cntm = int(m.sum())
                fs_idx[off:off + cntm] = es[m] - k * CLSR
                fd_idx[off:off + cntm] = ed[m]
                sc_idx[off:off + cntm] = ed[m]
                off += n
        cores.append((fs_idx, fd_idx, sc_idx))
    return cores, fs_segs, sc_segs, S, T_all


def _wrap16(idx):
    w = np.ascontiguousarray(np.asarray(idx).reshape(-1, 16).T).astype(np.int16)
    return np.tile(w, (8, 1))


def _pieces(segs, t0, t1):
    """Intersect segment list [(off, n, *rest)] with slot range [t0*128, t1*128)."""
    a, b = t0 * 128, t1 * 128
    out = []
    for s in segs:
        off, n = s[0], s[1]
        lo, hi = max(off, a), min(off + n, b)
        if lo < hi:
            out.append((lo, hi - lo) + tuple(s[2:]))
    return out


def _build(prep, pos_cnt, zero_cnt):
    import concourse.bass as bass
    import concourse.tile as tile
    from concourse import bacc, mybir, library_config

    cores, fs_segs, sc_segs, S, T_all = prep
    ALPHA = 0.2

    nc = bacc.Bacc("TRN2", target_bir_lowering=False, debug=False,
                   num_devices=NCORES)
    dt = mybir.dt
    atomT_d = nc.dram_tensor("atomT", [ATOM_DIM + 1, NPC], dt.bfloat16,
                             kind="ExternalInput")
    win_d = nc.dram_tensor("win", [ATOM_DIM + 1, HID], dt.bfloat16,
                           kind="ExternalInput")
    wsd_d = nc.dram_tensor("wsd", [128, 2 * LAYERS, 512], dt.bfloat16,
                           kind="ExternalInput")
    fsi_d = nc.dram_tensor("fsi", [128, S // 16], dt.int16, kind="ExternalInput")
    fdi_d = nc.dram_tensor("fdi", [128, S // 16], dt.int16, kind="ExternalInput")
    sci_d = nc.dram_tensor("sci", [128, S // 16], dt.int16, kind="ExternalInput")
    ident_d = nc.dram_tensor("ident", [128, 128], dt.bfloat16, kind="ExternalInput")
    out_d = nc.dram_tensor("out", [NPC, HID], dt.float32, kind="ExternalOutput")

    fs_bounce = nc.dram_tensor("fs_bounce", [NPC, HID], dt.bfloat16)
    fs_full = nc.dram_tensor("fs_full", [N, HID], dt.bfloat16,
                             addr_space="Shared")
    fd_tab = nc.dram_tensor("fd_tab", [NPC, HID], dt.bfloat16)
    rst = nc.dram_tensor("rst", [NACC, PAY], dt.bfloat16)
    rst2 = nc.dram_tensor("rst2", [NACC, PAY], dt.bfloat16)

    sc_par_segs = [(off, n, i % 2) for i, (off, n) in enumerate(sc_segs)]
    NB = 98            # 128-node blocks (last has 84 valid)
    n_chunks = (T_all + CHUNK - 1) // CHUNK

    with tile.TileContext(nc) as tc:
        nc.gpsimd.load_library(library_config.mlp)
        with tc.tile_pool(name="persist", bufs=1) as pp, \
             tc.tile_pool(name="work", bufs=3) as wp, \
             tc.tile_pool(name="payp", bufs=4) as yp, \
             tc.tile_pool(name="mono", bufs=1) as mp, \
             tc.tile_pool(name="stage", bufs=2) as sp, \
             tc.tile_pool(name="psA", bufs=2, space="PSUM") as psA, \
             tc.tile_pool(name="psT", bufs=2, space="PSUM") as psT:

            fsi = pp.tile([128, S // 16], dt.int16, tag="fsi")
            fdi = pp.tile([128, S // 16], dt.int16, tag="fdi")
            sci = pp.tile([128, S // 16], dt.int16, tag="sci")
            wsd = pp.tile([128, 2 * LAYERS, 512], dt.bfloat16, tag="wsd")
            win = pp.tile([ATOM_DIM + 1, HID], dt.bfloat16, tag="win")
            ident = pp.tile([128, 128], dt.bfloat16, tag="ident")
            hT = pp.tile([128, 2, NB * 128], dt.bfloat16, tag="hT")
            nc.sync.dma_start(fsi[:], fsi_d[:])
            nc.sync.dma_start(fdi[:], fdi_d[:])
            nc.sync.dma_start(sci[:], sci_d[:])
            nc.sync.dma_start(wsd[:], wsd_d[:])
            nc.sync.dma_start(win[:], win_d[:])
            nc.sync.dma_start(ident[:], ident_d[:])

            # ---- input projection: hT0 = (atomT^T @ Win)^T directly
            # atomT shares its memory (tag) with zpay: atomT is dead after
            # the input projection, before the first zpay memset runs.
            atomT = mp.tile([ATOM_DIM + 1, NPC], dt.bfloat16, tag="big")
            nc.sync.dma_start(atomT[:], atomT_d[:])
            NS = 25
            W_ = NPC // NS  # 500
            for s in range(NS):
                for cch in range(2):
                    ps = psA.tile([128, 512], dt.float32, tag="projps")
                    nc.tensor.matmul(ps[:, 0:W_],
                                     win[:, cch * 128:(cch + 1) * 128],
                                     atomT[:, s * W_:(s + 1) * W_],
                                     start=True, stop=True)
                    nc.scalar.activation(out=hT[:, cch, s * W_:(s + 1) * W_],
                                         in_=ps[:, 0:W_],
                                         func=mybir.ActivationFunctionType.Copy)

            zpay = mp.tile([128, 14, PAY], dt.bfloat16, tag="big")
            nc.vector.memset(zpay[:], 0.0)

            for l in range(LAYERS):
                last = l == LAYERS - 1
                # ---- projection: fs/fd tables
                for a in range(NB):
                    nt = 128 if a < NB - 1 else NPC - 128 * (NB - 1)
                    j = a % 8
                    if j == 0:
                        fs_sb = sp.tile([128, 8, HID], dt.bfloat16, tag="fs_sb")
                        fd_sb = sp.tile([128, 8, HID], dt.bfloat16, tag="fd_sb")
                    ps = psA.tile([128, 512], dt.float32, tag="projps")
                    for kc in range(2):
                        nc.tensor.matmul(
                            ps[0:nt, :],
                            hT[:, kc, a * 128:a * 128 + nt],
                            wsd[:, l * 2 + kc, :],
                            start=(kc == 0), stop=(kc == 1))
                    nc.scalar.activation(out=fs_sb[0:nt, j, :], in_=ps[0:nt, 0:HID],
                                         func=mybir.ActivationFunctionType.Copy)
                    nc.scalar.activation(out=fd_sb[0:nt, j, :], in_=ps[0:nt, HID:512],
                                         func=mybir.ActivationFunctionType.Copy)
                    if j == 7 or a == NB - 1:
                        a0 = a - j
                        fullc = j + (1 if nt == 128 else 0)
                        if fullc:
                            nc.sync.dma_start(
                                fs_bounce[a0 * 128:(a0 + fullc) * 128, :].rearrange(
                                    "(a p) e -> p a e", p=128),
                                fs_sb[:, 0:fullc, :])
                            nc.sync.dma_start(
                                fd_tab[a0 * 128:(a0 + fullc) * 128, :].rearrange(
                                    "(a p) e -> p a e", p=128),
                                fd_sb[:, 0:fullc, :])
                        if nt < 128:
                            nc.sync.dma_start(
                                fs_bounce[(NB - 1) * 128:NPC, :],
                                fs_sb[0:nt, j, :])
                            nc.sync.dma_start(
                                fd_tab[(NB - 1) * 128:NPC, :],
                                fd_sb[0:nt, j, :])

                # ---- AllGather fs table
                nc.gpsimd.collective_compute(
                    "AllGather", mybir.AluOpType.bypass,
                    replica_groups=[list(range(NCORES))],
                    ins=[fs_bounce[:].opt()], outs=[fs_full[:].opt()])

                # ---- zero-init accumulators (98 = 7 x 14 column groups)
                for acc in (rst, rst2):
                    for zk in range(7):
                        nc.sync.dma_start(
                            acc[zk * 14 * 128:(zk + 1) * 14 * 128, :].rearrange(
                                "(a p) e -> p a e", p=128), zpay[:])

                # ---- per-chunk edge pipeline
                for ch in range(n_chunks):
                    t0 = ch * CHUNK
                    t1 = min(t0 + CHUNK, T_all)
                    tcn = t1 - t0
                    fsg = wp.tile([128, CHUNK, HID], dt.bfloat16, tag="fsg")
                    fdg = wp.tile([128, CHUNK, HID], dt.bfloat16, tag="fdg")
                    pay = yp.tile([128, CHUNK, PAY], dt.bfloat16, tag="pay")
                    pn = wp.tile([128, CHUNK, 8], dt.float32, tag="pn")
                    lg = wp.tile([128, CHUNK, 4], dt.float32, tag="lg")
                    exb = wp.tile([128, CHUNK, 4], dt.bfloat16, tag="exb")

                    for (so, n, k) in _pieces(fs_segs, t0, t1):
                        hi = min((k + 1) * CLSR, N)
                        nc.gpsimd.dma_gather(
                            fsg[:, so // 128 - t0:(so + n) // 128 - t0, :],
                            fs_full[k * CLSR:hi, :],
                            fsi[:, so // 16:(so + n) // 16], n, n, HID)
                    nc.gpsimd.dma_gather(
                        fdg[:, 0:tcn, :], fd_tab[:],
                        fdi[:, t0 * 8:t1 * 8], tcn * 128, tcn * 128, HID)

                    u = fdg  # in-place: u = lrelu(fsg + fdg)
                    nc.vector.tensor_tensor(out=u[:, 0:tcn, :], in0=fsg[:, 0:tcn, :],
                                            in1=fdg[:, 0:tcn, :],
                                            op=mybir.AluOpType.add)
                    nc.scalar.activation(out=u[:, 0:tcn, :], in_=u[:, 0:tcn, :],
                                         func=mybir.ActivationFunctionType.Prelu,
                                         alpha=ALPHA)
                    for h in range(HEADS):
                        kp = int(pos_cnt[l, h])
                        kz = int(zero_cnt[l, h])
                        if kp > 0:
                            nc.vector.tensor_reduce(
                                out=pn[:, 0:tcn, h],
                                in_=u[:, 0:tcn, h * OUT:h * OUT + kp],
                                axis=mybir.AxisListType.X, op=mybir.AluOpType.add)
                        else:
                            nc.vector.memset(pn[:, 0:tcn, h], 0.0)
                        if kp + kz < OUT:
                            nc.vector.tensor_reduce(
                                out=pn[:, 0:tcn, 4 + h],
                                in_=u[:, 0:tcn, h * OUT + kp + kz:(h + 1) * OUT],
                                axis=mybir.AxisListType.X, op=mybir.AluOpType.add)
                        else:
                            nc.vector.memset(pn[:, 0:tcn, 4 + h], 0.0)
                    nc.vector.tensor_tensor(out=lg[:, 0:tcn, :],
                                            in0=pn[:, 0:tcn, 0:4],
                                            in1=pn[:, 0:tcn, 4:8],
                                            op=mybir.AluOpType.subtract)
                    nc.scalar.activation(out=exb[:, 0:tcn, :], in_=lg[:, 0:tcn, :],
                                         func=mybir.ActivationFunctionType.Exp)
                    nc.vector.memset(pay[:, 0:tcn, HID + HEADS:PAY], 0.0)
                    nc.vector.tensor_copy(out=pay[:, 0:tcn, HID:HID + HEADS],
                                          in_=exb[:, 0:tcn, :])
                    nc.vector.tensor_tensor(
                        out=pay[:, 0:tcn, 0:HID].rearrange(
                            "p t (h d) -> p t h d", h=HEADS),
                        in0=fsg[:, 0:tcn, :].rearrange(
                            "p t (h d) -> p t h d", h=HEADS),
                        in1=exb[:, 0:tcn, :].unsqueeze(3).broadcast_to(
                            [128, tcn, HEADS, OUT]),
                        op=mybir.AluOpType.mult)
                    for (so0, n0, rpar) in _pieces(sc_par_segs, t0, t1):
                        acc = rst if rpar == 0 else rst2
                        for so in range(so0, so0 + n0, 640):
                            n = min(640, so0 + n0 - so)
                            nc.gpsimd.dma_scatter_add(
                                acc[:],
                                pay[:, so // 128 - t0:(so + n) // 128 - t0, :],
                                sci[:, so // 16:(so + n) // 16], n, n, PAY)

                # ---- readback + normalize (+ transpose for next layer)
                for g0 in range(0, NB, 8):
                    gn = min(8, NB - g0)
                    rb = sp.tile([128, 8, PAY], dt.bfloat16, tag="fs_sb")
                    rb2 = sp.tile([128, 8, PAY], dt.bfloat16, tag="rb2")
                    nc.sync.dma_start(
                        rb[:, 0:gn, :],
                        rst[g0 * 128:(g0 + gn) * 128, :].rearrange(
                            "(a p) e -> p a e", p=128))
                    nc.sync.dma_start(
                        rb2[:, 0:gn, :],
                        rst2[g0 * 128:(g0 + gn) * 128, :].rearrange(
                            "(a p) e -> p a e", p=128))
                    nc.vector.tensor_tensor(out=rb[:, 0:gn, :], in0=rb[:, 0:gn, :],
                                            in1=rb2[:, 0:gn, :],
                                            op=mybir.AluOpType.add)
                    hn = sp.tile([128, 8, HID],
                                 dt.float32 if last else dt.bfloat16, tag="fd_sb")
                    denf = wp.tile([128, 32], dt.float32, tag="denf")
                    rec = wp.tile([128, 32], dt.float32, tag="rec")
                    nc.vector.tensor_scalar(
                        out=denf[:, 0:gn * 4].rearrange("p (a b) -> p a b", b=4),
                        in0=rb[:, 0:gn, HID:HID + HEADS],
                        scalar1=EPS, scalar2=None, op0=mybir.AluOpType.add)
                    nc.vector.reciprocal(out=rec[:, 0:gn * 4], in_=denf[:, 0:gn * 4])
                    for j in range(gn):
                        for h in range(HEADS):
                            nc.vector.tensor_scalar(
                                out=hn[:, j, h * OUT:(h + 1) * OUT],
                                in0=rb[:, j, h * OUT:(h + 1) * OUT],
                                scalar1=rec[:, j * 4 + h:j * 4 + h + 1], scalar2=None,
                                op0=mybir.AluOpType.mult)
                    if last:
                        nn = min(NPC - g0 * 128, gn * 128)
                        if nn == gn * 128:
                            nc.sync.dma_start(
                                out_d[g0 * 128:g0 * 128 + nn, :].rearrange(
                                    "(a p) e -> p a e", p=128),
                                hn[:, 0:gn, :])
                        else:
                            fullc = nn // 128
                            if fullc:
                                nc.sync.dma_start(
                                    out_d[g0 * 128:(g0 + fullc) * 128, :].rearrange(
                                        "(a p) e -> p a e", p=128),
                                    hn[:, 0:fullc, :])
                            rem = nn - fullc * 128
                            nc.sync.dma_start(
                                out_d[(g0 + fullc) * 128:(g0 + fullc) * 128 + rem, :],
                                hn[0:rem, fullc, :])
                    else:
                        for j in range(gn):
                            a = g0 + j
                            for cch in range(2):
                                pt = psT.tile([128, 128], dt.bfloat16, tag="tp")
                                nc.tensor.transpose(
                                    pt[:], hn[:, j, cch * 128:(cch + 1) * 128],
                                    ident[:])
                                nc.vector.tensor_copy(
                                    out=hT[:, cch, a * 128:(a + 1) * 128],
                                    in_=pt[:])
    nc.compile()
    return nc


def kernel(**inputs):
    from concourse.bass_utils import run_bass_kernel_spmd

    src = np.asarray(inputs['src'])
    dst = np.asarray(inputs['dst'])
    atom = np.asarray(inputs['atom_feat']).astype(np.float32)
    Ws_eff, Wd_eff, bs_eff, bd_eff, pos_cnt, zero_cnt, T2inv = _fold_weights(
        inputs['W_in'], inputs['b_in'], inputs['W_src'], inputs['b_src'],
        inputs['W_dst'], inputs['b_dst'], inputs['attn'], inputs['bias'])
    prep = _prep_slots(src, dst)
    cores, fs_segs, sc_segs, S, T_all = prep

    # weight tensors
    win_np = np.zeros((ATOM_DIM + 1, HID), np.float32)
    win_np[:ATOM_DIM] = np.asarray(inputs['W_in'])
    win_np[ATOM_DIM] = np.asarray(inputs['b_in'])
    wsd_np = np.zeros((128, 2 * LAYERS, 512), np.float32)
    for l in range(LAYERS):
        for kc in range(2):
            wsd_np[:, l * 2 + kc, 0:HID] = Ws_eff[l][kc * 128:(kc + 1) * 128]
            wsd_np[:, l * 2 + kc, HID:512] = Wd_eff[l][kc * 128:(kc + 1) * 128]
    # biases are folded via ones-row only for input proj; bs_eff/bd_eff assumed 0
    # (setup_inputs uses zero biases). Guard:
    for l in range(LAYERS):
        assert np.abs(bs_eff[l]).max() < 1e-12 and np.abs(bd_eff[l]).max() < 1e-12, \
            "nonzero GAT biases not supported by this kernel build"

    nc = _build(prep, pos_cnt, zero_cnt)

    ident = np.eye(128, dtype=np.float32)
    in_maps = []
    for c in range(NCORES):
        fs_idx, fd_idx, sc_idx = cores[c]
        at = np.zeros((ATOM_DIM + 1, NPC), np.float32)
        at[:ATOM_DIM] = atom[c * NPC:(c + 1) * NPC].T
        at[ATOM_DIM] = 1.0
        in_maps.append({
            'atomT': _bf(at), 'win': _bf(win_np), 'wsd': _bf(wsd_np),
            'fsi': _wrap16(fs_idx), 'fdi': _wrap16(fd_idx),
            'sci': _wrap16(sc_idx), 'ident': _bf(ident),
        })
    import os
    res = run_bass_kernel_spmd(nc, in_maps, core_ids=list(range(NCORES)),
                               trace=bool(os.environ.get('KBT_TRACE')))
    kernel._last = res
    out = np.concatenate([res.results[c]['out'] for c in range(NCORES)], 0)
    out = out.astype(np.float64) @ T2inv + np.asarray(inputs['bias'])[LAYERS - 1][None]
    return out.astype(np.float32)


if __name__ == '__main__':
    import jax
    with jax.default_device(jax.devices('cpu')[0]):
        import reference
        inputs = {k: np.asarray(v) for k, v in reference.setup_inputs().items()}
    got = kernel(**inputs)
    print("kernel out:", got.shape, got.dtype, np.abs(got).mean())



# revision 6
# speedup vs baseline: 2.6262x; 2.0503x over previous
"""Trainium2 8-core GATv2 message-passing kernel (nn_AtomGraphEncoder).

Design (v2 — PE-onehot):
- Nodes block-sharded 8x12500, degree-balanced permutation into 98 windows
  of 128 nodes per core; edges assigned to dst's core.
- Per layer: project fs/fd from node-major h table (PE, with on-the-fly
  transposes), AllGather fs into a Shared HBM table, then per group of 4
  windows: dma_gather fs rows (the only GPSIMD-heavy op), expand fd per
  edge-slot via one-hot matmuls (PE), alpha-folded prelu (ACT) + head
  reduces (DVE) for GATv2 logits, exp, msg = ex*fs, and segment-sum
  aggregation via transposed one-hot matmuls into per-window PSUM (PE) —
  no dma_scatter_add, no HBM accumulator round-trip.
- |a| and the 0.2 lrelu factor are folded into W columns on host
  (pos: a, neg: -0.2|a| with alpha=5 prelu), undone on output.
"""
import sys
import os

import numpy as np
import ml_dtypes

sys.path.insert(0, '/opt/trn_rl_repo')
sys.path.insert(0, os.path.dirname(os.path.abspath(__file__)))

N, E = 100000, 400000
ATOM_DIM, HID, LAYERS, HEADS = 74, 256, 3, 4
OUT = HID // HEADS
NCORES = 8
NPC = N // NCORES            # 12500
CLS, CLSR = 4, 32768
NWIN = (NPC + 127) // 128    # 98
G = 4
NGRP = (NWIN + G - 1) // G   # 25
BF = ml_dtypes.bfloat16
EPS = 1e-20
ALPHA = 0.2

# ---------------------------------------------------------------------------
# host prep (inlined so kernel.py is self-contained for the harness)


def _fold_weights(W_src, b_src, W_dst, b_dst, attn, bias):
    Ts, Tinvs = [], []
    pos_cnt = np.zeros((LAYERS, HEADS), np.int64)
    zero_cnt = np.zeros((LAYERS, HEADS), np.int64)
    for l in range(LAYERS):
        Tl = np.zeros((HID, HID), np.float64)
        Tinv = np.zeros((HID, HID), np.float64)
        for h in range(HEADS):
            a = np.asarray(attn)[l, h].astype(np.float64)
            order = np.concatenate([
                np.where(a > 0)[0], np.where(a == 0)[0], np.where(a < 0)[0]])
            pos_cnt[l, h] = (a > 0).sum()
            zero_cnt[l, h] = (a == 0).sum()
            for j, p in enumerate(order):
                if a[p] > 0:
                    s = a[p]
                elif a[p] == 0:
                    s = 1.0
                else:
                    s = -ALPHA * abs(a[p])
                Tl[h * OUT + p, h * OUT + j] = s
                Tinv[h * OUT + j, h * OUT + p] = 1.0 / s
        Ts.append(Tl)
        Tinvs.append(Tinv)
    Ws_eff, Wd_eff, bs_eff, bd_eff = [], [], [], []
    for l in range(LAYERS):
        Tp = np.eye(HID) if l == 0 else Tinvs[l - 1]
        Ws = np.asarray(W_src)[l].astype(np.float64)
        Wd = np.asarray(W_dst)[l].astype(np.float64)
        bprev = np.zeros(HID) if l == 0 else np.asarray(bias)[l - 1].astype(np.float64)
        Ws_eff.append((Tp @ Ws @ Ts[l]).astype(np.float32))
        Wd_eff.append((Tp @ Wd @ Ts[l]).astype(np.float32))
        bs_eff.append(((np.asarray(b_src)[l] + bprev @ Ws) @ Ts[l]).astype(np.float32))
        bd_eff.append(((np.asarray(b_dst)[l] + bprev @ Wd) @ Ts[l]).astype(np.float32))
    return Ws_eff, Wd_eff, bs_eff, bd_eff, pos_cnt, zero_cnt, Tinvs[-1]


def _balanced_perm(deg):
    caps = np.full(NWIN, 128, np.int64)
    caps[-1] = NPC - 128 * (NWIN - 1)
    order = np.argsort(-deg, kind="stable")
    fill = np.zeros(NWIN, np.int64)
    r_of_node = np.empty(NPC, np.int64)
    seq = np.concatenate([np.arange(NWIN), np.arange(NWIN)[::-1]])
    ptr = 0
    for node in order:
        while fill[seq[ptr % (2 * NWIN)]] >= caps[seq[ptr % (2 * NWIN)]]:
            ptr += 1
        w = seq[ptr % (2 * NWIN)]
        r_of_node[node] = w * 128 + fill[w]
        fill[w] += 1
        ptr += 1
    nodes_by_r = np.empty(NPC, np.int64)
    nodes_by_r[r_of_node] = np.arange(NPC)
    return r_of_node, nodes_by_r


def _prep(src, dst):
    src = np.asarray(src).astype(np.int64)
    dst = np.asarray(dst).astype(np.int64)

    cores_edges = []
    r_of_node_all = []
    nodes_by_r_all = []
    for c in range(NCORES):
        m = (dst >= c * NPC) & (dst < (c + 1) * NPC)
        es, dl = src[m], dst[m] - c * NPC
        deg = np.bincount(dl, minlength=NPC)
        r_of_node, nodes_by_r = _balanced_perm(deg)
        cores_edges.append((es, r_of_node[dl]))
        r_of_node_all.append(r_of_node)
        nodes_by_r_all.append(nodes_by_r)

    fsrow = np.empty(N, np.int64)
    for c in range(NCORES):
        fsrow[c * NPC:(c + 1) * NPC] = c * NPC + r_of_node_all[c]

    cnt = np.zeros((NCORES, NWIN, CLS), np.int64)
    core_wk = []
    for c in range(NCORES):
        es, r_d = cores_edges[c]
        w = r_d // 128
        k = fsrow[es] // CLSR
        np.add.at(cnt[c], (w, k), 1)
        core_wk.append((w, k))
    seg = cnt.max(axis=0)

    run_base = np.zeros((NWIN, CLS), np.int64)
    pieces = [[] for _ in range(NGRP)]
    group_cols = []
    off = 0
    for g in range(NGRP):
        wins = list(range(g * G, min(NWIN, g * G + G)))
        g0 = off
        for k in range(CLS):
            cstart = off
            for w in wins:
                run_base[w, k] = off
                off += seg[w, k]
            off += (-(off - cstart)) % 128
            if off > cstart:
                pieces[g].append((k, cstart, off - cstart))
        group_cols.append((g0 // 128, off // 128))
    S = off
    T_all = S // 128

    pairs = [[] for _ in range(NGRP)]
    PM = np.full((T_all, G), -1, np.int64)
    pi = 0
    for g in range(NGRP):
        wins = list(range(g * G, min(NWIN, g * G + G)))
        c0, c1 = group_cols[g]
        plist = []
        for col in range(c0, c1):
            a, b = col * 128, (col + 1) * 128
            for wl, w in enumerate(wins):
                for k in range(CLS):
                    lo = max(run_base[w, k], a)
                    hi = min(run_base[w, k] + seg[w, k], b)
                    if lo < hi:
                        plist.append((col - c0, wl))
                        break
        seen = set()
        plist2 = []
        for p in plist:
            if p not in seen:
                seen.add(p)
                plist2.append(p)
        wl_first, wl_last, col_first, col_last = {}, {}, {}, {}
        for i, (cl, wl) in enumerate(plist2):
            wl_first.setdefault(wl, i)
            wl_last[wl] = i
            col_first.setdefault(cl, i)
            col_last[cl] = i
        for i, (cl, wl) in enumerate(plist2):
            pairs[g].append(dict(
                c=cl, w=wl, pi=pi,
                e_start=(col_first[cl] == i), e_stop=(col_last[cl] == i),
                a_start=(wl_first[wl] == i), a_stop=(wl_last[wl] == i)))
            PM[c0 + cl, wl] = pi
            pi += 1
        for w in range(len(wins)):
            assert w in wl_first, f"window {g * G + w} has no pairs"
    P_total = pi
    Pmax = max(len(p) for p in pairs)
    Cmax = max(c1 - c0 for c0, c1 in group_cols)

    cores = []
    for c in range(NCORES):
        es, r_d = cores_edges[c]
        w, k = core_wk[c]
        order = np.lexsort((r_d, k, w))
        es_s, rd_s, w_s, k_s = es[order], r_d[order], w[order], k[order]
        key = w_s * CLS + k_s
        uniq, starts = np.unique(key, return_index=True)
        rank = np.arange(len(key)) - np.repeat(
            starts, np.diff(np.concatenate([starts, [len(key)]])))
        slot = run_base[w_s, k_s] + rank
        row = fsrow[es_s]
        fs_idx = np.zeros(S, np.int64)
        fs_idx[slot] = row - k_s * CLSR
        g_s = w_s // G
        wl_s = w_s - g_s * G
        pi_e = PM[slot // 128, wl_s]
        assert (pi_e >= 0).all()
        ohA = np.zeros((128, P_total, 128), BF)
        ohE = np.zeros((128, P_total, 128), BF)
        ohA[slot % 128, pi_e, rd_s % 128] = 1.0
        ohE[rd_s % 128, pi_e, slot % 128] = 1.0
        cores.append(dict(fs_idx=fs_idx, ohA=ohA, ohE=ohE))

    return dict(seg=seg, run_base=run_base, pieces=pieces, pairs=pairs,
                group_cols=group_cols, S=S, T_all=T_all, P_total=P_total,
                Pmax=Pmax, Cmax=Cmax, cores=cores,
                nodes_by_r=nodes_by_r_all)


def _bf(x):
    return np.asarray(x).astype(BF)


def _wrap16(idx):
    w = np.ascontiguousarray(np.asarray(idx).reshape(-1, 16).T).astype(np.int16)
    return np.tile(w, (8, 1))


# ---------------------------------------------------------------------------
# bass build


def _build(P, pos_cnt):
    import concourse.bass as bass
    import concourse.tile as tile
    from concourse import bacc, mybir, library_config

    S = P['S']
    Pmax, Cmax, P_total = P['Pmax'], P['Cmax'], P['P_total']

    nc = bacc.Bacc("TRN2", target_bir_lowering=False, debug=False,
                   num_devices=NCORES)
    dt = mybir.dt
    atomT_d = nc.dram_tensor("atomT", [ATOM_DIM + 1, NPC], dt.bfloat16,
                             kind="ExternalInput")
    win_d = nc.dram_tensor("win", [ATOM_DIM + 1, HID], dt.bfloat16,
                           kind="ExternalInput")
    wsd_d = nc.dram_tensor("wsd", [128, 2 * LAYERS, 512], dt.bfloat16,
                           kind="ExternalInput")
    fsi_d = nc.dram_tensor("fsi", [128, S // 16], dt.int16, kind="ExternalInput")
    ohE_d = nc.dram_tensor("ohE", [128, P_total, 128], dt.bfloat16,
                           kind="ExternalInput")
    ohA_d = nc.dram_tensor("ohA", [128, P_total, 128], dt.bfloat16,
                           kind="ExternalInput")
    ident_d = nc.dram_tensor("ident", [128, 128], dt.bfloat16, kind="ExternalInput")
    out_d = nc.dram_tensor("out", [NPC, HID], dt.float32, kind="ExternalOutput")

    fs_bounce = nc.dram_tensor("fs_bounce", [NPC, HID], dt.bfloat16)
    fs_full = nc.dram_tensor("fs_full", [N, HID], dt.bfloat16,
                             addr_space="Shared")

    LASTN = NPC - 128 * (NWIN - 1)   # 84

    with tile.TileContext(nc) as tc:
        nc.gpsimd.load_library(library_config.mlp)
        with tc.tile_pool(name="persist", bufs=1) as pp, \
             tc.tile_pool(name="atp", bufs=3) as ap_, \
             tc.tile_pool(name="htp", bufs=3) as hp, \
             tc.tile_pool(name="stage", bufs=2) as sp, \
             tc.tile_pool(name="ohp", bufs=2) as op_, \
             tc.tile_pool(name="fsgp", bufs=2) as wp, \
             tc.tile_pool(name="upp", bufs=2) as up_, \
             tc.tile_pool(name="payp", bufs=2) as yp, \
             tc.tile_pool(name="smallp", bufs=2) as mp, \
             tc.tile_pool(name="psE", bufs=2, space="PSUM") as psE, \
             tc.tile_pool(name="psT", bufs=2, space="PSUM") as psT, \
             tc.tile_pool(name="aggp", bufs=1, space="PSUM") as aggp:

            fsi = pp.tile([128, S // 16], dt.int16, tag="fsi")
            wsd = pp.tile([128, 2 * LAYERS, 512], dt.bfloat16, tag="wsd")
            win = pp.tile([ATOM_DIM + 1, HID], dt.bfloat16, tag="win")
            ident = pp.tile([128, 128], dt.bfloat16, tag="ident")
            fdh = pp.tile([128, NWIN, HID], dt.bfloat16, tag="fdh")
            nc.sync.dma_start(fsi[:], fsi_d[:])
            nc.sync.dma_start(wsd[:], wsd_d[:])
            nc.sync.dma_start(win[:], win_d[:])
            nc.sync.dma_start(ident[:], ident_d[:])

            # ---- input projection: fdh = h0 (node-major, permuted order)
            nc.vector.memset(fdh[:, NWIN - 1, :], 0.0)
            for a in range(NWIN):
                nt = 128 if a < NWIN - 1 else LASTN
                at = ap_.tile([ATOM_DIM + 1, 128], dt.bfloat16, tag="at")
                nc.sync.dma_start(at[:, 0:nt], atomT_d[:, a * 128:a * 128 + nt])
                ps = psE.tile([128, 512], dt.float32, tag="pse")
                nc.tensor.matmul(ps[0:nt, 0:HID], at[:, 0:nt], win[:],
                                 start=True, stop=True)
                nc.scalar.activation(out=fdh[0:nt, a, :], in_=ps[0:nt, 0:HID],
                                     func=mybir.ActivationFunctionType.Copy)

            for l in range(LAYERS):
                last = l == LAYERS - 1
                # ---- projection phase: fs -> fs_bounce (HBM), fd -> fdh
                for a in range(NWIN):
                    nt = 128 if a < NWIN - 1 else LASTN
                    hTst = hp.tile([128, 2, 128], dt.bfloat16, tag="hTst")
                    for cch in range(2):
                        pt = psT.tile([128, 128], dt.bfloat16, tag="tp")
                        nc.tensor.transpose(
                            pt[:], fdh[:, a, cch * 128:(cch + 1) * 128], ident[:])
                        nc.scalar.activation(
                            out=hTst[:, cch, :], in_=pt[:],
                            func=mybir.ActivationFunctionType.Copy)
                    ps = psE.tile([128, 512], dt.float32, tag="pse")
                    for kc in range(2):
                        nc.tensor.matmul(
                            ps[0:nt, :], hTst[:, kc, 0:nt],
                            wsd[:, l * 2 + kc, :],
                            start=(kc == 0), stop=(kc == 1))
                    j = a % 8
                    if j == 0:
                        fs_sb = sp.tile([128, 8, HID], dt.bfloat16, tag="fs_sb")
                    nc.scalar.activation(out=fs_sb[0:nt, j, :], in_=ps[0:nt, 0:HID],
                                         func=mybir.ActivationFunctionType.Copy)
                    nc.scalar.activation(out=fdh[0:nt, a, :], in_=ps[0:nt, HID:512],
                                         func=mybir.ActivationFunctionType.Copy)
                    if j == 7 or a == NWIN - 1:
                        a0 = a - j
                        fullc = j + (1 if nt == 128 else 0)
                        if fullc:
                            nc.sync.dma_start(
                                fs_bounce[a0 * 128:(a0 + fullc) * 128, :].rearrange(
                                    "(a p) e -> p a e", p=128),
                                fs_sb[:, 0:fullc, :])
                        if nt < 128:
                            nc.sync.dma_start(
                                fs_bounce[(NWIN - 1) * 128:NPC, :],
                                fs_sb[0:nt, j, :])

                # ---- AllGather fs table
                nc.gpsimd.collective_compute(
                    "AllGather", mybir.AluOpType.bypass,
                    replica_groups=[list(range(NCORES))],
                    ins=[fs_bounce[:].opt()], outs=[fs_full[:].opt()])

                # ---- edge phase: software-pipelined groups
                holdover = None   # (g, ohAt, pay, aggtiles...)
                for g in range(NGRP):
                    c0, c1 = P['group_cols'][g]
                    Cg = c1 - c0
                    wins = list(range(g * G, min(NWIN, g * G + G)))
                    pg = P['pairs'][g]
                    Pg = len(pg)
                    pbase = pg[0]['pi']

                    ohEt = op_.tile([128, Pmax, 128], dt.bfloat16, tag="ohE")
                    ohAt = op_.tile([128, Pmax, 128], dt.bfloat16, tag="ohA")
                    nc.sync.dma_start(ohEt[:, 0:Pg, :],
                                      ohE_d[:, pbase:pbase + Pg, :])
                    nc.sync.dma_start(ohAt[:, 0:Pg, :],
                                      ohA_d[:, pbase:pbase + Pg, :])
                    fsg = wp.tile([128, Cmax, HID], dt.bfloat16, tag="fsg")
                    for (k, soff, n) in P['pieces'][g]:
                        hi = min(N, (k + 1) * CLSR)
                        nc.gpsimd.dma_gather(
                            fsg[:, soff // 128 - c0:(soff + n) // 128 - c0, :],
                            fs_full[k * CLSR:hi, :],
                            fsi[:, soff // 16:(soff + n) // 16], n, n, HID)

                    # expand fd per slot-col + u = fs + fd
                    upre = up_.tile([128, Cmax, HID], dt.bfloat16, tag="upre")
                    percol = {}
                    for i, pr in enumerate(pg):
                        percol.setdefault(pr['c'], []).append(i)
                    for cl in range(Cg):
                        ps = psE.tile([128, 512], dt.float32, tag="pse")
                        idxs = percol[cl]
                        for ii, i in enumerate(idxs):
                            pr = pg[i]
                            nc.tensor.matmul(
                                ps[:, 0:HID], ohEt[:, i, :],
                                fdh[:, wins[pr['w']], :],
                                start=(ii == 0), stop=(ii == len(idxs) - 1))
                        nc.vector.tensor_tensor(
                            out=upre[:, cl, :], in0=ps[:, 0:HID],
                            in1=fsg[:, cl, :], op=mybir.AluOpType.add)

                    # alpha-folded prelu + per-head logits
                    for h in range(HEADS):
                        kp = int(pos_cnt[l, h])
                        if kp:
                            nc.scalar.activation(
                                out=upre[:, 0:Cg, h * OUT:h * OUT + kp],
                                in_=upre[:, 0:Cg, h * OUT:h * OUT + kp],
                                func=mybir.ActivationFunctionType.Prelu,
                                alpha=ALPHA)
                        if kp < OUT:
                            nc.scalar.activation(
                                out=upre[:, 0:Cg, h * OUT + kp:(h + 1) * OUT],
                                in_=upre[:, 0:Cg, h * OUT + kp:(h + 1) * OUT],
                                func=mybir.ActivationFunctionType.Prelu,
                                alpha=1.0 / ALPHA)
                    lg = mp.tile([128, Cmax, 4], dt.float32, tag="lg")
                    for h in range(HEADS):
                        nc.vector.tensor_reduce(
                            out=lg[:, 0:Cg, h],
                            in_=upre[:, 0:Cg, h * OUT:(h + 1) * OUT],
                            axis=mybir.AxisListType.X, op=mybir.AluOpType.add)
                    exb = mp.tile([128, Cmax, 4], dt.bfloat16, tag="exb")
                    nc.scalar.activation(out=exb[:, 0:Cg, :], in_=lg[:, 0:Cg, :],
                                         func=mybir.ActivationFunctionType.Exp)
                    pay = yp.tile([128, Cmax, HID + 4], dt.bfloat16, tag="pay")
                    nc.vector.tensor_tensor(
                        out=pay[:, 0:Cg, 0:HID].rearrange(
                            "p t (h d) -> p t h d", h=HEADS),
                        in0=fsg[:, 0:Cg, :].rearrange(
                            "p t (h d) -> p t h d", h=HEADS),
                        in1=exb[:, 0:Cg, :].unsqueeze(3).broadcast_to(
                            [128, Cg, HEADS, OUT]),
                        op=mybir.AluOpType.mult)
                    nc.vector.tensor_copy(out=pay[:, 0:Cg, HID:HID + 4],
                                          in_=exb[:, 0:Cg, :])

                    # aggregate + normalize the PREVIOUS group (pipeline)
                    if holdover is not None:
                        _agg_norm(nc, tile, mybir, P, pos_cnt, aggp, mp, sp,
                                  fdh, out_d, holdover, last)
                    holdover = (g, wins, pg, ohAt, pay)
                if holdover is not None:
                    _agg_norm(nc, tile, mybir, P, pos_cnt, aggp, mp, sp,
                              fdh, out_d, holdover, last)
    nc.compile()
    return nc


def _agg_norm(nc, tile, mybir, P, pos_cnt, aggp, mp, sp, fdh, out_d,
              holdover, last):
    dt = mybir.dt
    g, wins, pg, ohAt, pay = holdover
    cur = {}
    outst = None
    if last:
        outst = sp.tile([128, G, HID], dt.float32, tag="outst")
    for i, pr in enumerate(pg):
        wl = pr['w']
        if pr['a_start']:
            cur[wl] = aggp.tile([128, HID + 4], dt.float32, tag=f"agg{wl}",
                                name=f"aggps{wl}")
        psA = cur[wl]
        nc.tensor.matmul(psA[:, 0:HID + 4], ohAt[:, i, :],
                         pay[:, pr['c'], 0:HID + 4],
                         start=pr['a_start'], stop=pr['a_stop'])
        if pr['a_stop']:
            W = wins[wl]
            denf = mp.tile([128, 4], dt.float32, tag="denf")
            rec = mp.tile([128, 4], dt.float32, tag="rec")
            nc.vector.tensor_scalar(out=denf[:], in0=psA[:, HID:HID + 4],
                                    scalar1=EPS, scalar2=None,
                                    op0=mybir.AluOpType.add)
            nc.vector.reciprocal(out=rec[:], in_=denf[:])
            for h in range(HEADS):
                dst = (outst[:, wl, h * OUT:(h + 1) * OUT] if last
                       else fdh[:, W, h * OUT:(h + 1) * OUT])
                nc.vector.tensor_scalar(
                    out=dst, in0=psA[:, h * OUT:(h + 1) * OUT],
                    scalar1=rec[:, h:h + 1], scalar2=None,
                    op0=mybir.AluOpType.mult)
    if last:
        LASTN = NPC - 128 * (NWIN - 1)
        w0 = wins[0]
        fullw = len(wins) if wins[-1] < NWIN - 1 else len(wins) - 1
        if fullw:
            nc.sync.dma_start(
                out_d[w0 * 128:(w0 + fullw) * 128, :].rearrange(
                    "(a p) e -> p a e", p=128),
                outst[:, 0:fullw, :])
        if wins[-1] == NWIN - 1:
            nc.sync.dma_start(
                out_d[(NWIN - 1) * 128:NPC, :],
                outst[0:LASTN, len(wins) - 1, :])


# ---------------------------------------------------------------------------


def kernel(**inputs):
    from concourse.bass_utils import run_bass_kernel_spmd

    src = np.asarray(inputs['src'])
    dst = np.asarray(inputs['dst'])
    atom = np.asarray(inputs['atom_feat']).astype(np.float32)
    Ws_eff, Wd_eff, bs_eff, bd_eff, pos_cnt, zero_cnt, T2inv = _fold_weights(
        inputs['W_src'], inputs['b_src'], inputs['W_dst'], inputs['b_dst'],
        inputs['attn'], inputs['bias'])
    for l in range(LAYERS):
        assert np.abs(bs_eff[l]).max() < 1e-12 and np.abs(bd_eff[l]).max() < 1e-12, \
            "nonzero GAT biases not supported by this kernel build"
    assert (zero_cnt == 0).all(), "zero attention weights not supported"

    P = _prep(src, dst)

    win_np = np.zeros((ATOM_DIM + 1, HID), np.float32)
    win_np[:ATOM_DIM] = np.asarray(inputs['W_in'])
    win_np[ATOM_DIM] = np.asarray(inputs['b_in'])
    wsd_np = np.zeros((128, 2 * LAYERS, 512), np.float32)
    for l in range(LAYERS):
        for kc in range(2):
            wsd_np[:, l * 2 + kc, 0:HID] = Ws_eff[l][kc * 128:(kc + 1) * 128]
            wsd_np[:, l * 2 + kc, HID:512] = Wd_eff[l][kc * 128:(kc + 1) * 128]

    nc = _build(P, pos_cnt)

    ident = np.eye(128, dtype=np.float32)
    in_maps = []
    for c in range(NCORES):
        cd = P['cores'][c]
        at = np.zeros((ATOM_DIM + 1, NPC), np.float32)
        at[:ATOM_DIM] = atom[c * NPC + P['nodes_by_r'][c]].T
        at[ATOM_DIM] = 1.0
        in_maps.append({
            'atomT': _bf(at), 'win': _bf(win_np), 'wsd': _bf(wsd_np),
            'fsi': _wrap16(cd['fs_idx']),
            'ohE': cd['ohE'], 'ohA': cd['ohA'],
            'ident': _bf(ident),
        })
    res = run_bass_kernel_spmd(nc, in_maps, core_ids=list(range(NCORES)),
                               trace=bool(os.environ.get('KBT_TRACE')))
    kernel._last = res
    full = np.empty((N, HID), np.float64)
    for c in range(NCORES):
        full[c * NPC + P['nodes_by_r'][c]] = res.results[c]['out']
    full = full @ T2inv + np.asarray(inputs['bias'])[LAYERS - 1][None]
    return full.astype(np.float32)


if __name__ == '__main__':
    import jax
    with jax.default_device(jax.devices('cpu')[0]):
        import reference
        inputs = {k: np.asarray(v) for k, v in reference.setup_inputs().items()}
    got = kernel(**inputs)
    print("kernel out:", got.shape, got.dtype, np.abs(got).mean())
